# revision 1
# baseline (speedup 1.0000x reference)
"""Trainium2 Bass kernel for nn_CrossAttention1d (B=8, C=768, N=256, H=12, D=64).

Math (per batch b), algebraically equal to the reference but avoiding the
[3072, 3072] attention matrix via associativity:

    cp  = W_proj @ cross_b + b_proj                  [C, N]
    CP  = cp.reshape(D, H*N)      (pure reshape)
    Xc  = cross_b.reshape(D, H*N) (pure reshape)
    K   = CP @ Xc^T                                  [D, D]
    X   = x_ori_b.reshape(D, H*N)
    OT  = scale * K^T @ X                            [D, H*N]   (= O^T)
    out2T[h*64+d, n] = OT[d, n*12+h]                 [C, N]
    yT  = W_dep @ out2T + b_dep                      [C, N]
    out_b = x_ori_b + yT

Sharding: data-parallel over batch, one batch per NeuronCore (8 cores).

Implementation notes (per core / per iteration):
  - bf16 on the compute path (fp8 quantization of any compute tensor costs
    2-3e-2 rel error alone, over the 2e-2 budget).  The residual copy of x
    is fp8: its error is damped ~16x by the output magnitude (2e-3).
  - cross^T (K-matmul rhs) is a host-side permutation of cross and is DMA'd,
    killing the baseline's 12 PE transposes + 12 evictions.
  - 4 input DMAs: combined [128, 12296] bf16 tensor (cross | cross^T |
    W_proj^T | W_dep^T | b_dep columns), x as [64, 3072] bf16, fp8 residual
    [128, 1536], b_proj [1, 768].  One bf16 store.  Input DMAs issue on the
    SP queue, the store on the Pool queue, so a store waiting on compute
    never head-blocks the next set's loads.
  - Four-stage software pipeline across NSETS=4 rotating buffer sets:
    the loop body emits P(s)=DMA+proj, Q(s-1)=K, R(s-2)=OT, S(s-3)=deproj
    +residual+store.  The PE queue is in-order, so putting a full pipeline
    slot between a PSUM eviction and its consumer removes all cross-stage
    stalls (measured ~10 us/iter of stalls with naive stage order).
  - HW-measured: matmuls sustain 0.367 ns/row at full clock with ~48 ns
    fixed overhead each; DMA sustains ~307 GB/s/core under 8-core load.
  - b_dep is added via scalar_tensor_tensor per-partition bias during the
    deproj eviction (out = (yps + bd) + xr), saving 6 rank-1 matmuls.
  - PSUM evictions: GPSIMD/Pool cannot read PSUM on real HW, so Activation
    takes cpT+K and DVE takes OT + deproj evictions.
"""

import numpy as np

import concourse.bacc as bacc
import concourse.mybir as mybir
import concourse.tile as tile
from concourse.bass_utils import run_bass_kernel_spmd

B, C, N = 8, 768, 256
H, D = 12, 64
M = H * N  # 3072
SCALE = float(D) ** -0.5
N_CORES = 8
F32 = mybir.dt.float32
BF16 = mybir.dt.bfloat16
FP8 = mybir.dt.float8e4

NSETS = 4  # pipeline depth == number of stages

# big-tensor column offsets
O_CROSS = 0
O_CRT = 1536
O_WP = 3072
O_WD = 7680
O_BD = 12288
BIGW = 12296

_built_nc = None
Copy = mybir.ActivationFunctionType.Copy
ADD = mybir.AluOpType.add


def stage_P(nc, ctx, ones, ppj):
    """Input DMAs + proj: cpT[n, o] = (cross^T Wp^T + bp)[n, o]."""
    big, xp, xr, bias, out = ctx["dram"]
    nc.sync.dma_start(ctx["big_sb"][:], big.ap())
    nc.sync.dma_start(ctx["xp_sb"][:], xp.ap())
    nc.sync.dma_start(ctx["xr_sb"][:], xr.ap())
    nc.sync.dma_start(ctx["bias_sb"][:], bias.ap())
    big_sb, cpT = ctx["big_sb"], ctx["cpT"]
    for ni in range(2):
        for oj in range(2):
            ps = ppj.tile([128, 384], F32, name="ps")
            for t in range(6):
                nc.tensor.matmul(
                    ps[:],
                    big_sb[:, O_CROSS + t * N + ni * 128:
                           O_CROSS + t * N + ni * 128 + 128],
                    big_sb[:, O_WP + t * C + oj * 384:
                           O_WP + t * C + oj * 384 + 384],
                    start=(t == 0), stop=False,
                )
            nc.tensor.matmul(
                ps[:], ones[0:1, 0:128],
                ctx["bias_sb"][0:1, oj * 384:(oj + 1) * 384],
                start=False, stop=True,
            )
            nc.scalar.activation(
                cpT[:, ni * C + oj * 384: ni * C + oj * 384 + 384],
                ps[:], Copy)


def stage_Q(nc, ctx, pk):
    """K[d', d] = sum_{ni,h,p} cpT[p, ni, d'*12+h] crT[p, ni, d*12+h]."""
    big_sb, cpT = ctx["big_sb"], ctx["cpT"]
    kps = pk.tile([64, 64], F32, name="kps")
    first = True
    for ni in range(2):
        for h in range(H):
            nc.tensor.matmul(
                kps[:],
                cpT[:, ni * C + h: ni * C + h + 63 * 12 + 1: 12],
                big_sb[:, O_CRT + ni * C + h:
                       O_CRT + ni * C + h + 63 * 12 + 1: 12],
                start=first, stop=(ni == 1 and h == H - 1),
            )
            first = False
    nc.scalar.activation(ctx["k_sb"][:], kps[:], Copy, scale=SCALE)


def stage_R(nc, ctx, pot):
    """OT: po[p6*64+d, u] = OT[d, m], m = 2r+p6+6u; de-interleave to ot2."""
    xp_sb, k_sb, ot2 = ctx["xp_sb"], ctx["k_sb"], ctx["ot2"]
    for r in range(3):
        po = pot.tile([128, 512], F32, name="po")
        nc.tensor.matmul(po[0:64, :], k_sb[:], xp_sb[:, 2 * r::6],
                         start=True, stop=True)
        nc.tensor.matmul(po[64:128, :], k_sb[:], xp_sb[:, 2 * r + 1::6],
                         start=True, stop=True)
        src = po[:].rearrange("p (n s) -> p s n", s=2)
        dst = ot2[:, r * 256: r * 256 + 1024].rearrange(
            "p (s n) -> p s n", s=4)[:, 0::3]
        nc.vector.tensor_copy(dst, src)


def stage_S(nc, ctx, py):
    """deproj + b_dep + residual, store."""
    big_sb, ot2, xr_sb, out_sb = (
        ctx["big_sb"], ctx["ot2"], ctx["xr_sb"], ctx["out_sb"])
    for oi in range(6):
        yps = py.tile([128, 256], F32, name="yps")
        for q in range(6):
            nc.tensor.matmul(
                yps[:],
                big_sb[:, O_WD + q * C + oi * 128: O_WD + q * C + oi * 128 + 128],
                ot2[:, q * 256:(q + 1) * 256],
                start=(q == 0), stop=(q == 5),
            )
        # out = (yps + bd[c']) + xr
        nc.vector.scalar_tensor_tensor(
            out_sb[:, oi * 256:(oi + 1) * 256],
            yps[:], big_sb[:, O_BD + oi: O_BD + oi + 1],
            xr_sb[:, oi * 256:(oi + 1) * 256],
            ADD, ADD,
        )
    nc.gpsimd.dma_start(ctx["dram"][4].ap(), out_sb[:])


def _declare(nc, n_sets):
    """Inputs are shared across pipeline slots (read-only); out is per-slot."""
    big = nc.dram_tensor("big", [128, BIGW], BF16, kind="ExternalInput")
    xp = nc.dram_tensor("xp", [64, M], BF16, kind="ExternalInput")
    xr = nc.dram_tensor("xr", [128, 1536], FP8, kind="ExternalInput")
    bias = nc.dram_tensor("bias", [1, 768], BF16, kind="ExternalInput")
    args = []
    for s in range(n_sets):
        sfx = f"_{s}" if n_sets > 1 else ""
        out = nc.dram_tensor(f"out{sfx}", [128, 1536], BF16, kind="ExternalOutput")
        args.append((big, xp, xr, bias, out))
    return args


def _pools(tc, nc, n_sets):
    const = tc.alloc_tile_pool(name="const", bufs=1)
    ones = const.tile([1, 256], BF16)
    nc.gpsimd.memset(ones[:], 1.0)
    sbd = tc.alloc_tile_pool(name="sbd", bufs=n_sets)
    ppj = tc.alloc_tile_pool(name="ppj", bufs=2, space="PSUM")
    pk = tc.alloc_tile_pool(name="pk", bufs=2, space="PSUM")
    pot = tc.alloc_tile_pool(name="pot", bufs=2, space="PSUM")
    py = tc.alloc_tile_pool(name="py", bufs=2, space="PSUM")
    pools = (const, sbd, ppj, pk, pot, py)
    return pools, ones, sbd, (ppj, pk, pot, py)


def _make_ctx(sbd, dram):
    return {
        "dram": dram,
        "big_sb": sbd.tile([128, BIGW], BF16, name="big_sb"),
        "xp_sb": sbd.tile([64, M], BF16, name="xp_sb"),
        "xr_sb": sbd.tile([128, 1536], FP8, name="xr_sb"),
        "bias_sb": sbd.tile([1, 768], BF16, name="bias_sb"),
        "cpT": sbd.tile([128, 1536], BF16, name="cpT"),
        "k_sb": sbd.tile([64, 64], BF16, name="k_sb"),
        "ot2": sbd.tile([128, 1536], BF16, name="ot2"),
        "out_sb": sbd.tile([128, 1536], BF16, name="out_sb"),
    }


def build():
    nc = bacc.Bacc("TRN2", target_bir_lowering=False, debug=False)
    args = _declare(nc, 1)
    with tile.TileContext(nc) as tc:
        pools, ones, sbd, (ppj, pk, pot, py) = _pools(tc, nc, 1)
        ctx = _make_ctx(sbd, args[0])
        stage_P(nc, ctx, ones, ppj)
        stage_Q(nc, ctx, pk)
        stage_R(nc, ctx, pot)
        stage_S(nc, ctx, py)
        for p in reversed(pools):
            p.release()
    nc.compile()
    return nc


def build_flat(n_iters):
    """n_iters unrolled (no hardware loop) - for timeline sim."""
    nc = bacc.Bacc("TRN2", target_bir_lowering=False, debug=False)
    args = _declare(nc, NSETS)
    with tile.TileContext(nc) as tc:
        pools, ones, sbd, (ppj, pk, pot, py) = _pools(tc, nc, NSETS)
        ctxs = [_make_ctx(sbd, args[s]) for s in range(NSETS)]
        for it in range(n_iters + 3):
            if it < n_iters:
                stage_P(nc, ctxs[it % NSETS], ones, ppj)
            if 1 <= it:
                stage_Q(nc, ctxs[(it - 1) % NSETS], pk)
            if 2 <= it:
                stage_R(nc, ctxs[(it - 2) % NSETS], pot)
            if 3 <= it:
                stage_S(nc, ctxs[(it - 3) % NSETS], py)
        for p in reversed(pools):
            p.release()
    nc.compile()
    return nc


def build_loop(reps):
    """Kernel body wrapped in a hardware For loop, for wall-clock timing.

    Steady-state wrap: stage X(s-k) reads tiles written k body-positions
    earlier (possibly the previous trip).  All trips use identical input
    data, so the final trip's outputs are exact.
    """
    assert reps % NSETS == 0, f"reps must be divisible by {NSETS}"
    nc = bacc.Bacc("TRN2", target_bir_lowering=False, debug=False)
    args = _declare(nc, NSETS)
    with tile.TileContext(nc) as tc:
        pools, ones, sbd, (ppj, pk, pot, py) = _pools(tc, nc, NSETS)
        ctxs = [_make_ctx(sbd, args[s]) for s in range(NSETS)]
        with tc.For_i(0, reps // NSETS, 1, hint_engines=(mybir.EngineType.PE,)):
            for s in range(NSETS):
                stage_P(nc, ctxs[s], ones, ppj)
                stage_Q(nc, ctxs[(s - 1) % NSETS], pk)
                stage_R(nc, ctxs[(s - 2) % NSETS], pot)
                stage_S(nc, ctxs[(s - 3) % NSETS], py)
        for p in reversed(pools):
            p.release()
    nc.compile()
    return nc


def make_in_maps(x_ori, cross, W_proj, b_proj, W_dep, b_dep):
    import ml_dtypes

    fp8 = ml_dtypes.float8_e4m3
    bf16 = ml_dtypes.bfloat16
    x_ori = np.asarray(x_ori, np.float32)
    cross = np.asarray(cross, np.float32)

    def w_perm(w):  # [o, c] -> [128, (t o)] of W^T
        return w.T.reshape(6, 128, C).transpose(1, 0, 2).reshape(128, 4608)

    wpP = w_perm(np.asarray(W_proj, np.float32))
    wdP = w_perm(np.asarray(W_dep, np.float32))
    bdT = np.asarray(b_dep, np.float32).reshape(6, 128).T  # [128, 6]
    bdT = np.concatenate([bdT, np.zeros((128, 2), np.float32)], axis=1)
    biasP = np.asarray(b_proj, np.float32).reshape(1, 768).astype(bf16)

    maps = []
    for b in range(B):
        cr, xo = cross[b], x_ori[b]
        crossP = cr.reshape(6, 128, N).transpose(1, 0, 2).reshape(128, 1536)
        crT = cr.T.reshape(2, 128, C).transpose(1, 0, 2).reshape(128, 1536)
        big = np.ascontiguousarray(
            np.concatenate([crossP, crT, wpP, wdP, bdT], axis=1)).astype(bf16)
        xpP = np.ascontiguousarray(xo.reshape(D, M)).astype(bf16)
        xrP = np.ascontiguousarray(
            xo.reshape(6, 128, N).transpose(1, 0, 2).reshape(128, 1536)
        ).astype(fp8)
        maps.append({"big": big, "xp": xpP, "xr": xrP, "bias": biasP})
    return maps


def unpermute_out(o):  # [128, (u n)] -> [C, N]
    return np.asarray(o, np.float32).reshape(128, 6, N).transpose(1, 0, 2).reshape(C, N)


def kernel(**inputs):
    global _built_nc
    if _built_nc is None:
        _built_nc = build()
    nc = _built_nc
    in_maps = make_in_maps(
        inputs["x_ori"], inputs["cross"], inputs["W_proj"],
        inputs["b_proj"], inputs["W_dep"], inputs["b_dep"],
    )
    res = run_bass_kernel_spmd(nc, in_maps, list(range(N_CORES)))
    out = np.stack([unpermute_out(res.results[c]["out"]) for c in range(N_CORES)])
    return out.astype(np.float32)

